# revision 2
# baseline (speedup 1.0000x reference)
"""Trainium2 Bass kernel for disparity cost-volume construction — v3.

Layout: H sharded over 8 cores, partition index p=(b,c,hb), per-core
DRAM outputs [128, D, 1200]. Changes vs v2 (199-243us, bimodal):

Traces showed SDMA engine 15 intermittently degrades to ~22 GB/s
(vs 26.8 line rate on engines 0-14), and the bufs=2 pipeline then
stalls all engines on its completions: 240us runs. A probe kernel
showed HWDGE assigns descriptor i of a dma_start over partitions
[s:e] to engine (i-s) mod 16 — always starting at engine 0 — so a
transfer of <=15 partitions never touches engine 15.

1. d=0 plane written DRAM->DRAM straight from the input tensors
   (no SBUF dependency: output stream starts ~3us earlier).
2. d=52..63 written in "15-mode": 9 chunks of <=15 partitions per
   dma_start, skipping engine 15. Engine 15 keeps ~81% of an even
   share, matching its degraded rate; engines 0-14 absorb the rest.
3. Inputs load first on both rings, then d0, then ramped batches.
"""

from contextlib import ExitStack

import numpy as np

B, C, H, W, D = 2, 32, 80, 240, 64
NCORES = 8
HL = H // NCORES  # local rows per core (10)
HB, H5 = 2, 5  # local rows split: 2 partition groups x 5 rows
P = B * C * HB  # 128 partitions
F = H5 * W  # 1200 free elements per (partition, d)

# disparity batches: d=0 direct from DRAM, then ramp 1,2,4, then 8s
BATCHES = []
_d = 1
for nd in (1, 2, 4, 8, 8, 8, 8, 8, 8, 8):
    BATCHES.append((_d, nd))
    _d += nd
assert _d == D, _d

D15 = 52  # d >= D15 go out in 15-partition chunks (skip engine 15)

# partition chunking that avoids engine 15: positions 0..14 per chunk
CHUNKS15 = [(s, min(s + 15, P)) for s in range(0, P, 15)]

_CACHE: dict = {}


def _build():
    if "nc" in _CACHE:
        return _CACHE["nc"]

    import concourse.bacc as bacc
    import concourse.mybir as mybir
    import concourse.tile as tile

    f32 = mybir.dt.float32
    nc = bacc.Bacc("TRN2", target_bir_lowering=False, debug=False)

    x_t = nc.dram_tensor("x", [P, F], f32, kind="ExternalInput")
    y_t = nc.dram_tensor("y", [P, F], f32, kind="ExternalInput")
    ol_t = nc.dram_tensor("out_l", [P, D, F], f32, kind="ExternalOutput")
    or_t = nc.dram_tensor("out_r", [P, D, F], f32, kind="ExternalOutput")

    with tile.TileContext(nc) as tc, ExitStack() as ctx:
        inpool = ctx.enter_context(tc.tile_pool(name="inp", bufs=1))
        lpool = ctx.enter_context(tc.tile_pool(name="lt", bufs=2))
        rpool = ctx.enter_context(tc.tile_pool(name="rt", bufs=2))

        x_sb = inpool.tile([P, F], f32)
        y_sb = inpool.tile([P, F], f32)
        nc.sync.dma_start(x_sb, x_t.ap())
        nc.scalar.dma_start(y_sb, y_t.ap())
        xv = x_sb.rearrange("p (h w) -> p h w", h=H5)
        yv = y_sb.rearrange("p (h w) -> p h w", h=H5)

        # d=0: left is x verbatim, right is y verbatim — DRAM->DRAM,
        # no SBUF dependency, streams while the loads land
        nc.sync.dma_start(ol_t.ap()[:, 0:1, :], x_t.ap())
        nc.scalar.dma_start(or_t.ap()[:, 0:1, :], y_t.ap())

        for db, nd in BATCHES:
            lt = lpool.tile([P, nd * F], f32, tag="lt")
            rt = rpool.tile([P, nd * F], f32, tag="rt")
            ltv = lt.rearrange("p (j h w) -> p j h w", j=nd, h=H5)
            rtv = rt.rearrange("p (j h w) -> p j h w", j=nd, h=H5)
            for j in range(nd):
                d = db + j
                nc.vector.memset(ltv[:, j, :, 0:d], 0.0)
                nc.vector.memset(rtv[:, j, :, 0:d], 0.0)
                nc.vector.tensor_copy(ltv[:, j, :, d:W], xv[:, :, d:W])
                nc.scalar.copy(rtv[:, j, :, d:W], yv[:, :, 0 : W - d])
            # split the DMA at the d >= D15 boundary: full-width below,
            # 15-partition chunks (engine-15-free) at and above
            lo = min(max(D15 - db, 0), nd)  # first lo d's go full-width
            if lo > 0:
                nc.sync.dma_start(
                    ol_t.ap()[:, db : db + lo, :], lt[:, 0 : lo * F]
                )
                nc.scalar.dma_start(
                    or_t.ap()[:, db : db + lo, :], rt[:, 0 : lo * F]
                )
            if lo < nd:
                for ps, pe in CHUNKS15:
                    nc.sync.dma_start(
                        ol_t.ap()[ps:pe, db + lo : db + nd, :],
                        lt[ps:pe, lo * F : nd * F],
                    )
                    nc.scalar.dma_start(
                        or_t.ap()[ps:pe, db + lo : db + nd, :],
                        rt[ps:pe, lo * F : nd * F],
                    )

    nc.compile()
    _CACHE["nc"] = nc
    return nc


def _shard_inputs(x: np.ndarray, y: np.ndarray):
    x = np.asarray(x, dtype=np.float32)
    y = np.asarray(y, dtype=np.float32)
    in_maps = []
    for k in range(NCORES):
        xs = np.ascontiguousarray(x[:, :, k * HL : (k + 1) * HL, :]).reshape(P, F)
        ys = np.ascontiguousarray(y[:, :, k * HL : (k + 1) * HL, :]).reshape(P, F)
        in_maps.append({"x": xs, "y": ys})
    return in_maps


def _gather(results) -> np.ndarray:
    full = np.empty((B, 2 * C, D, H, W), dtype=np.float32)
    for k in range(NCORES):
        h0 = k * HL
        for name, c0 in (("out_l", 0), ("out_r", C)):
            shard = (
                results[k][name]
                .reshape(B, C, HB, D, H5, W)
                .transpose(0, 1, 3, 2, 4, 5)
                .reshape(B, C, D, HL, W)
            )
            full[:, c0 : c0 + C, :, h0 : h0 + HL, :] = shard
    return full


def _run(x: np.ndarray, y: np.ndarray, trace: bool = False):
    from concourse.bass_utils import run_bass_kernel_spmd

    nc = _build()
    in_maps = _shard_inputs(x, y)
    res = run_bass_kernel_spmd(
        nc, in_maps, core_ids=list(range(NCORES)), trace=trace
    )
    return _gather(res.results), res


def kernel(x: np.ndarray, y: np.ndarray) -> np.ndarray:
    out, _ = _run(x, y, trace=False)
    return out


# revision 16
# speedup vs baseline: 1.3689x; 1.3689x over previous
"""Trainium2 Bass kernel for disparity cost-volume construction.

Reference op: cost[:, :C, i, :, i:] = x[:, :, :, i:]
              cost[:, C:, i, :, i:] = y[:, :, :, :W-i]   (i = 0..D-1)
Output [B, 2C, D, H, W] = 629 MB fp32; inputs 12.6 MB — the kernel is
pure HBM-write-bandwidth bound (memory regime).

Sharding: H axis over 8 cores (no halo). Per core the partition index
is p = (b, c, hb) with hb in {0,1} covering 5 rows each -> 128
partitions x 1200 free elements; per-core DRAM outputs [128, D, 1200]
(left / right halves as separate tensors, gathered on host).

Schedule (per core, both HWDGE rings):
  - x loads on the SP ring, y on the ACT ring (parallel).
  - d=0 planes are written DRAM->DRAM straight from the input tensors
    (no SBUF dependency: the output stream starts ~3us earlier).
  - d=1..63 staged in SBUF in ramped batches (1,2,4 then 8s): vector
    masks/copies the left tiles, scalar the right tiles, then one
    full-width [128 x nd*1200] dma_start per ring per batch
    (38.4 KB/partition descriptors, 8 descriptors per SDMA engine per
    dma — enough to amortize the per-(dma x engine) completion stall).

Measured on trn2 (NTFF, core 0): ~200 us when the DMA subsystem is
healthy (all 16 SDMA engines at the ~27 GB/s port line rate; the
429 GB/s aggregate is within 2% of the 435 GB/s SBUF-AXI ceiling,
with ~14 us of fixed runtime barrier/spin-up/teardown); ~230-245 us
when the environment degrades SDMA engine 15 (or all engines) to
~22 GB/s — an interference mode that kernel-level rebalancing cannot
dodge: descriptor->engine assignment is positional (probe-verified),
so excluding engine 15 requires <=15-partition transfers, and those
pay a ~2-4 us per-(dma x engine) completion stall that costs more
than the straggler (measured: aligned 15-partition chunks run at
6.9 GB/s vs 27 GB/s full-width).

SBUF/partition: 2 pools x bufs=2 x 38.4 KB + 9.6 KB inputs = 163 KB.
"""

from contextlib import ExitStack

import numpy as np

B, C, H, W, D = 2, 32, 80, 240, 64
NCORES = 8
HL = H // NCORES  # local rows per core (10)
HB, H5 = 2, 5  # local rows split: 2 partition groups x 5 rows
P = B * C * HB  # 128 partitions
F = H5 * W  # 1200 free elements per (partition, d)

# disparity batches: d=0 direct from DRAM, then ramp 1,2,4, then 8s
BATCHES = []
_d = 1
for _nd in (1, 2, 4, 8, 8, 8, 8, 8, 8, 8):
    BATCHES.append((_d, _nd))
    _d += _nd
assert _d == D, _d

_CACHE: dict = {}


def _build():
    if "nc" in _CACHE:
        return _CACHE["nc"]

    import concourse.bacc as bacc
    import concourse.mybir as mybir
    import concourse.tile as tile

    f32 = mybir.dt.float32
    nc = bacc.Bacc("TRN2", target_bir_lowering=False, debug=False)

    x_t = nc.dram_tensor("x", [P, F], f32, kind="ExternalInput")
    y_t = nc.dram_tensor("y", [P, F], f32, kind="ExternalInput")
    ol_t = nc.dram_tensor("out_l", [P, D, F], f32, kind="ExternalOutput")
    or_t = nc.dram_tensor("out_r", [P, D, F], f32, kind="ExternalOutput")

    with tile.TileContext(nc) as tc, ExitStack() as ctx:
        inpool = ctx.enter_context(tc.tile_pool(name="inp", bufs=1))
        lpool = ctx.enter_context(tc.tile_pool(name="lt", bufs=2))
        rpool = ctx.enter_context(tc.tile_pool(name="rt", bufs=2))

        x_sb = inpool.tile([P, F], f32)
        y_sb = inpool.tile([P, F], f32)
        nc.sync.dma_start(x_sb, x_t.ap())
        nc.scalar.dma_start(y_sb, y_t.ap())
        xv = x_sb.rearrange("p (h w) -> p h w", h=H5)
        yv = y_sb.rearrange("p (h w) -> p h w", h=H5)

        # d=0: left is x verbatim, right is y verbatim — DRAM->DRAM,
        # no SBUF dependency, drains while the loads land
        nc.sync.dma_start(ol_t.ap()[:, 0:1, :], x_t.ap())
        nc.scalar.dma_start(or_t.ap()[:, 0:1, :], y_t.ap())

        for db, nd in BATCHES:
            lt = lpool.tile([P, nd * F], f32, tag="lt")
            rt = rpool.tile([P, nd * F], f32, tag="rt")
            ltv = lt.rearrange("p (j h w) -> p j h w", j=nd, h=H5)
            rtv = rt.rearrange("p (j h w) -> p j h w", j=nd, h=H5)
            for j in range(nd):
                d = db + j
                nc.vector.memset(ltv[:, j, :, 0:d], 0.0)
                nc.vector.memset(rtv[:, j, :, 0:d], 0.0)
                nc.vector.tensor_copy(ltv[:, j, :, d:W], xv[:, :, d:W])
                nc.scalar.copy(rtv[:, j, :, d:W], yv[:, :, 0 : W - d])
            nc.sync.dma_start(ol_t.ap()[:, db : db + nd, :], lt)
            nc.scalar.dma_start(or_t.ap()[:, db : db + nd, :], rt)

    nc.compile()
    _CACHE["nc"] = nc
    return nc


def _shard_inputs(x: np.ndarray, y: np.ndarray):
    x = np.asarray(x, dtype=np.float32)
    y = np.asarray(y, dtype=np.float32)
    in_maps = []
    for k in range(NCORES):
        xs = np.ascontiguousarray(x[:, :, k * HL : (k + 1) * HL, :]).reshape(P, F)
        ys = np.ascontiguousarray(y[:, :, k * HL : (k + 1) * HL, :]).reshape(P, F)
        in_maps.append({"x": xs, "y": ys})
    return in_maps


def _gather(results) -> np.ndarray:
    full = np.empty((B, 2 * C, D, H, W), dtype=np.float32)
    for k in range(NCORES):
        h0 = k * HL
        for name, c0 in (("out_l", 0), ("out_r", C)):
            shard = (
                results[k][name]
                .reshape(B, C, HB, D, H5, W)
                .transpose(0, 1, 3, 2, 4, 5)
                .reshape(B, C, D, HL, W)
            )
            full[:, c0 : c0 + C, :, h0 : h0 + HL, :] = shard
    return full


def _run(x: np.ndarray, y: np.ndarray, trace: bool = False):
    from concourse.bass_utils import run_bass_kernel_spmd

    nc = _build()
    in_maps = _shard_inputs(x, y)
    res = run_bass_kernel_spmd(
        nc, in_maps, core_ids=list(range(NCORES)), trace=trace
    )
    return _gather(res.results), res


def kernel(x: np.ndarray, y: np.ndarray) -> np.ndarray:
    out, _ = _run(x, y, trace=False)
    return out
